# revision 1
# baseline (speedup 1.0000x reference)
"""Trainium2 Bass kernel for nn_Attention_85169201480311.

Dense transformer block: 3x (conv3x3 -> GroupNorm(1) -> exact GELU) projections,
8-head attention over 1024 tokens with relative-position bias, 1x1 out-conv.

Sharding: data-parallel over batch (8 samples -> 8 cores), params replicated.

Per-core program:
 - conv3x3 = 18 PSUM-accumulating bf16 matmuls (2 cin chunks x 9 taps) against a
   zero-padded [128, 2, 34, 34] SBUF image.
 - GroupNorm stats: bn_stats/bn_aggr per psum tile + gpsimd partition_all_reduce;
   affine+GELU fused into the PSUM eviction via ACT scale/bias operands.
 - attention in scores^T layout (m on partitions): both matmuls consume the
   native conv output layout, K=32 scores matmuls row-packed 4-up via
   tile_position. Softmax without max-subtraction; relative-position bias
   applied multiplicatively post-exp: exp(s*S)*exp(bias), exp(bias) precomputed
   host-side (bf16) and streamed.
 - attn@v: lhsT = [v_head | ones] (M=33), so psum row 32/96 carries the softmax
   denominator; two heads per psum tile via tile_position col packing. The
   reciprocal runs on a [128,16] respread of the sums (DMA repartition), and the
   normalizing multiply reads psum directly, writing the partition-aligned
   [c, n] layout the 1x1 out-conv consumes.
"""
import sys
for p in ('/opt/trn_rl_repo', '/root/.axon_site/_ro/trn_rl_repo'):
    if p not in sys.path:
        sys.path.insert(0, p)

import numpy as np
import ml_dtypes

import concourse.bass as bass
import concourse.tile as tile
from concourse import mybir, bacc, bass_isa
from concourse import bass_utils
from concourse.masks import make_identity

F32 = mybir.dt.float32
BF16 = mybir.dt.bfloat16
AF = mybir.ActivationFunctionType

IH = IW = 32
N = IH * IW          # 1024 tokens
C = 256
HEADS = 8
DH = 32
SCALE = 32 ** -0.5
EPS = 1e-6
B = 8
P = 128
NCHUNK = C // P      # 2 channel chunks

_cache = {}
DEBUG_TAPS = False


def _rel_index():
    coords = np.stack(np.meshgrid(np.arange(IH), np.arange(IW), indexing='ij')).reshape(2, -1)
    rel = coords[:, :, None] - coords[:, None, :]
    rel[0] += IH - 1
    rel[1] += IW - 1
    rel[0] *= 2 * IW - 1
    return rel.sum(0)  # [n, m] int


def build_nc():
    nc = bacc.Bacc('TRN2', target_bir_lowering=False)

    x_d = nc.dram_tensor("x", [NCHUNK, P, IH, IW], BF16, kind="ExternalInput")
    w_d = {}
    for nm in ("wq", "wk", "wv"):
        w_d[nm] = nc.dram_tensor(nm, [P, NCHUNK, 9, C], BF16, kind="ExternalInput")
    wout_d = nc.dram_tensor("wout", [P, NCHUNK, C], F32, kind="ExternalInput")
    vecs_d = nc.dram_tensor("vecs", [P, 14], F32, kind="ExternalInput")
    eb_d = nc.dram_tensor("ebias", [HEADS, 8, P, N], BF16, kind="ExternalInput")
    out_d = nc.dram_tensor("out", [P, NCHUNK, N], F32, kind="ExternalOutput")
    dbg = {}
    if DEBUG_TAPS:
        for nm, shp, dt in (("dbg_q", [P, NCHUNK, N], BF16), ("dbg_k", [P, NCHUNK, N], BF16),
                            ("dbg_v", [P, NCHUNK, N], BF16), ("dbg_va", [P, 8, 8, 34], BF16),
                            ("dbg_st", [P, 3, 4], F32), ("dbg_ao", [P, NCHUNK, N], F32)):
            dbg[nm] = nc.dram_tensor(nm, shp, dt, kind="ExternalOutput")

    # vecs cols: gq0,gq1,bq0,bq1, gk0,gk1,bk0,bk1, gv0,gv1,bv0,bv1, bout0,bout1
    VGCOL = {"wq": 0, "wk": 4, "wv": 8}

    with tile.TileContext(nc) as tc:
        with tc.tile_pool(name="const", bufs=1) as const, \
             tc.tile_pool(name="proj", bufs=1) as proj, \
             tc.tile_pool(name="stats", bufs=2) as stats_p, \
             tc.tile_pool(name="attn", bufs=1) as attn_p:

            xpad = const.tile([P, NCHUNK, IH + 2, IW + 2], BF16)
            nc.vector.memset(xpad[:], 0.0)
            for c in range(NCHUNK):
                nc.sync.dma_start(xpad[:, c, 1:IH + 1, 1:IW + 1], x_d[c])
            vecs = const.tile([P, 14], F32)
            nc.sync.dma_start(vecs[:], vecs_d[:])
            eps_t = const.tile([P, 1], F32)
            nc.vector.memset(eps_t[:], EPS)
            wout_sb = const.tile([P, NCHUNK, C], F32)
            nc.sync.dma_start(wout_sb[:], wout_d[:])
            ident = const.tile([P, P], BF16)
            make_identity(nc, ident[:])

            q_sb = proj.tile([P, NCHUNK, N], BF16)
            k_sb = proj.tile([P, NCHUNK, N], BF16)
            v_bf = proj.tile([P, NCHUNK, N], BF16)
            v_aug = proj.tile([P, 8, 8, 34], BF16)   # [*, i, h, 0:32 v | 32 ones | pad]
            attn_out = attn_p.tile([P, NCHUNK, N], F32)
            out_sb = attn_p.tile([P, NCHUNK, N], F32)

            # ---------------- projections ----------------
            with tc.tile_pool(name="wpool", bufs=1) as wpool, \
                 tc.tile_pool(name="psc", bufs=6, space="PSUM") as psc:
                # warm up the PE HAM clock gate while the weight DMAs stream in
                warm_sb = wpool.tile([P, 512], BF16)
                nc.vector.memset(warm_sb[:], 0.0)
                ps_w = psc.tile([P, 512], F32, name="warm_ps", tag="warm", bufs=1)
                for _ in range(30):
                    nc.tensor.matmul(ps_w[:], warm_sb[:, 0:P], warm_sb[:],
                                     start=True, stop=True)
                w_sb = {}
                for nm in ("wq", "wk", "wv"):
                    w_sb[nm] = wpool.tile([P, NCHUNK, 9, C], BF16, name=f"sb_{nm}")
                    nc.sync.dma_start(w_sb[nm][:], w_d[nm][:])

                for iproj, (nm, dst) in enumerate(
                        (("wq", q_sb), ("wk", k_sb), ("wv", v_bf))):
                    ps_t = [psc.tile([P, 512], F32, name=f"cv_{nm}_{m}_{j}", tag="conv")
                            for m in range(2) for j in range(2)]
                    st_t = [stats_p.tile([P, 2, 6], F32, name=f"st_{nm}_{m}", tag="stats")
                            for m in range(2)]
                    for m in range(2):
                        for j in range(2):
                            pt = ps_t[m * 2 + j]
                            first = True
                            for c in range(NCHUNK):
                                for t in range(9):
                                    dy, dx = t // 3, t % 3
                                    rhs = xpad[:, c, 16 * j + dy:16 * j + dy + 16, dx:dx + 32]
                                    nc.tensor.matmul(
                                        pt[:], w_sb[nm][:, c, t, m * P:(m + 1) * P], rhs,
                                        start=first, stop=(c == NCHUNK - 1 and t == 8))
                                    first = False
                            nc.vector.bn_stats(out=st_t[m][:, j, :], in_=pt[:])
                    # per-channel mean/var over the 1024 spatial positions
                    mv = [stats_p.tile([P, 2], F32, name=f"mv_{nm}_{m}", tag="mv")
                          for m in range(2)]
                    for m in range(2):
                        nc.vector.bn_aggr(out=mv[m][:], in_=st_t[m][:])
                    prep = stats_p.tile([P, 4], F32, name=f"prep_{nm}", tag="prep")
                    for m in range(2):
                        nc.vector.tensor_copy(out=prep[:, 2 * m:2 * m + 1], in_=mv[m][:, 0:1])
                        sq = stats_p.tile([P, 1], F32, name=f"sq_{nm}_{m}", tag="sq")
                        nc.vector.tensor_mul(out=sq[:], in0=mv[m][:, 0:1], in1=mv[m][:, 0:1])
                        nc.vector.tensor_add(out=prep[:, 2 * m + 1:2 * m + 2],
                                             in0=mv[m][:, 1:2], in1=sq[:])
                    red = stats_p.tile([P, 4], F32, name=f"red_{nm}", tag="red")
                    nc.gpsimd.partition_all_reduce(red[:], prep[:], channels=P,
                                                   reduce_op=bass_isa.ReduceOp.add)
                    # mean = (c0+c2)/256 ; var = (c1+c3)/256 - mean^2
                    mt = stats_p.tile([P, 4], F32, name=f"mt_{nm}", tag="mt")
                    nc.vector.tensor_add(out=mt[:, 0:1], in0=red[:, 0:1], in1=red[:, 2:3])
                    nc.scalar.mul(out=mt[:, 0:1], in_=mt[:, 0:1], mul=1.0 / C)
                    nc.vector.tensor_add(out=mt[:, 1:2], in0=red[:, 1:2], in1=red[:, 3:4])
                    nc.scalar.mul(out=mt[:, 1:2], in_=mt[:, 1:2], mul=1.0 / C)
                    nc.vector.tensor_mul(out=mt[:, 2:3], in0=mt[:, 0:1], in1=mt[:, 0:1])
                    nc.vector.tensor_sub(out=mt[:, 1:2], in0=mt[:, 1:2], in1=mt[:, 2:3])
                    nc.scalar.activation(out=mt[:, 1:2], in_=mt[:, 1:2], func=AF.Sqrt,
                                         bias=eps_t[:], scale=1.0)
                    nc.vector.reciprocal(out=mt[:, 1:2], in_=mt[:, 1:2])
                    if DEBUG_TAPS:
                        nc.sync.dma_start(dbg["dbg_st"][:, iproj, :], mt[:])
                    gc = VGCOL[nm]
                    sc = stats_p.tile([P, 4], F32, name=f"sc_{nm}", tag="sc")
                    for m in range(2):
                        # s_m = g_m * rstd ; t_m = b_m - mean * s_m
                        nc.vector.tensor_mul(out=sc[:, m:m + 1],
                                             in0=vecs[:, gc + m:gc + m + 1], in1=mt[:, 1:2])
                        nc.vector.tensor_mul(out=sc[:, 2 + m:3 + m],
                                             in0=mt[:, 0:1], in1=sc[:, m:m + 1])
                        nc.vector.tensor_sub(out=sc[:, 2 + m:3 + m],
                                             in0=vecs[:, gc + 2 + m:gc + 3 + m],
                                             in1=sc[:, 2 + m:3 + m])
                    for m in range(2):
                        for j in range(2):
                            nc.scalar.activation(
                                out=dst[:, m, 512 * j:512 * (j + 1)], in_=ps_t[m * 2 + j][:],
                                func=AF.Gelu, scale=sc[:, m:m + 1], bias=sc[:, 2 + m:3 + m])

                # v transpose into v_aug via PE transpose (psum slots shared w/ conv tag)
                nc.vector.memset(v_aug[:, :, :, 32:33], 1.0)
                for k in range(NCHUNK):
                    for i in range(8):
                        pvt = psc.tile([P, P], BF16, name=f"vt_{k}_{i}", tag="conv")
                        nc.tensor.transpose(pvt[:], v_bf[:, k, P * i:P * (i + 1)], ident[:])
                        nc.scalar.copy(out=v_aug[:, i, 4 * k:4 * k + 4, 0:32], in_=pvt[:])

            if DEBUG_TAPS:
                nc.sync.dma_start(dbg["dbg_q"][:], q_sb[:])
                nc.sync.dma_start(dbg["dbg_k"][:], k_sb[:])
                nc.sync.dma_start(dbg["dbg_v"][:], v_bf[:])
                nc.sync.dma_start(dbg["dbg_va"][:], v_aug[:])

            # ---------------- attention ----------------
            from contextlib import ExitStack
            with ExitStack() as stk:
                attnT_p = stk.enter_context(tc.tile_pool(name="attnT_p", bufs=4))
                ebuf = stk.enter_context(tc.tile_pool(name="ebuf", bufs=6))
                ebias_p = stk.enter_context(tc.tile_pool(name="ebias_p", bufs=6))
                rc_p = stk.enter_context(tc.tile_pool(name="rc", bufs=2))
                ps_sc = stk.enter_context(tc.tile_pool(name="ps_sc", bufs=2, space="PSUM"))
                ps_avp = stk.enter_context(tc.tile_pool(name="ps_avp", bufs=2, space="PSUM"))
                attnTs = {}

                def scores_chain(pair):
                    h0, h1 = 2 * pair, 2 * pair + 1
                    for h in (h0, h1):
                        attnTs[h] = attnT_p.tile([P, 8, N], BF16, name=f"attnT_{h}",
                                                 tag="attnT")
                    # interleave the two heads so their matmuls run concurrently
                    # in distinct PE row groups
                    for i in range(8):
                        ps_s = {}
                        for h in (h0, h1):
                            g, r = h // 4, h % 4
                            ps_s[h] = ps_sc.tile([P, N], F32, name=f"s_{h}_{i}", tag="sc")
                            for nj in range(2):
                                nc.tensor.matmul(
                                    ps_s[h][:, 512 * nj:512 * (nj + 1)],
                                    k_sb[32 * r:32 * r + 32, g, P * i:P * (i + 1)],
                                    q_sb[32 * r:32 * r + 32, g, 512 * nj:512 * (nj + 1)],
                                    start=True, stop=True, tile_position=(32 * r, 0))
                        for h in (h0, h1):
                            e_bf = ebuf.tile([P, N], BF16, name=f"e_{h}_{i}", tag="e")
                            nc.scalar.activation(out=e_bf[:], in_=ps_s[h][:], func=AF.Exp,
                                                 scale=SCALE)
                            eb_t = ebias_p.tile([P, N], BF16, name=f"eb_{h}_{i}", tag="eb")
                            nc.sync.dma_start(eb_t[:], eb_d[h, i])
                            nc.vector.tensor_mul(out=attnTs[h][:, i, :], in0=e_bf[:],
                                                 in1=eb_t[:])

                def av_norm(pair):
                    grp = pair // 2
                    h0, h1 = 2 * pair, 2 * pair + 1
                    ps_av = ps_avp.tile([P, N], F32, name=f"av_{pair}", tag="av")
                    # attn @ [v | 1]: head h at col group 64*(h%2); psum row
                    # 64*(h%2)+32 carries the softmax denominator
                    for nj in range(2):
                        for i in range(8):
                            for h in (h0, h1):
                                rv = h % 2
                                nc.tensor.matmul(
                                    ps_av[64 * rv:64 * rv + 33, 512 * nj:512 * (nj + 1)],
                                    v_aug[:, i, h, 0:33],
                                    attnTs[h][:, i, 512 * nj:512 * (nj + 1)],
                                    start=(i == 0), stop=(i == 7),
                                    tile_position=(0, 64 * rv))
                    # normalize the pair: recip of sums via [128,16] respread
                    srows = rc_p.tile([64, N], F32, name=f"sr_{pair}", tag="sr")
                    sp = rc_p.tile([P, 16], F32, name=f"sp_{pair}", tag="sp")
                    rrow = rc_p.tile([64, N], F32, name=f"rr_{pair}", tag="rr")
                    rcp_bc = rc_p.tile([P, N], F32, name=f"rb_{pair}", tag="rb")
                    for rv in range(2):
                        nc.vector.tensor_copy(out=srows[32 * rv:32 * rv + 1, :],
                                              in_=ps_av[64 * rv + 32:64 * rv + 33, :])
                        nc.sync.dma_start(out=sp[:, 8 * rv:8 * rv + 8],
                                          in_=srows[32 * rv:32 * rv + 1, :])
                    nc.vector.reciprocal(out=sp[:], in_=sp[:])
                    for rv in range(2):
                        nc.sync.dma_start(out=rrow[32 * rv:32 * rv + 1, :],
                                          in_=sp[:, 8 * rv:8 * rv + 8])
                        rowap = rrow[32 * rv:32 * rv + 1, :]
                        src = bass.AP(tensor=rowap.tensor, offset=rowap.offset,
                                      ap=[list(rowap.ap[0]), [0, 32]]
                                      + [list(d) for d in rowap.ap[1:]])
                        nc.sync.dma_start(out=rcp_bc[64 * rv:64 * rv + 32, :], in_=src)
                    for h in (h0, h1):
                        r, rv = h % 4, h % 2
                        nc.vector.tensor_mul(out=attn_out[32 * r:32 * r + 32, grp, :],
                                             in0=ps_av[64 * rv:64 * rv + 32, :],
                                             in1=rcp_bc[64 * rv:64 * rv + 32, :])

                # software pipeline: av/normalize of pair p-1 is emitted after the
                # scores/exp chain of pair p so the PE never head-of-line blocks
                scores_chain(0)
                for pair in range(1, 4):
                    scores_chain(pair)
                    av_norm(pair - 1)
                av_norm(3)

                if DEBUG_TAPS:
                    nc.sync.dma_start(dbg["dbg_ao"][:], attn_out[:])

                # ---------------- 1x1 out conv ----------------
                for m in range(2):
                    for j in range(2):
                        ps_o = ps_sc.tile([P, 512], F32, name=f"o_{m}_{j}", tag="sc")
                        for k in range(NCHUNK):
                            nc.tensor.matmul(ps_o[:], wout_sb[:, k, m * P:(m + 1) * P],
                                             attn_out[:, k, 512 * j:512 * (j + 1)],
                                             start=(k == 0), stop=(k == NCHUNK - 1))
                        nc.scalar.activation(out=out_sb[:, m, 512 * j:512 * (j + 1)],
                                             in_=ps_o[:], func=AF.Identity,
                                             bias=vecs[:, 12 + m:13 + m], scale=1.0)
                nc.sync.dma_start(out_d[:], out_sb[:])

    nc.compile()
    return nc


def _prep_shared(Wq, gq, bq, Wk, gk, bk, Wv, gv, bv, bias_table, Wout, bout):
    def wt(W):
        # [co, ci, 3, 3] -> [ci%128, ci//128, tap, co]
        return np.ascontiguousarray(
            W.astype(np.float32).transpose(1, 2, 3, 0).reshape(NCHUNK, P, 9, C)
            .transpose(1, 0, 2, 3)).astype(ml_dtypes.bfloat16)
    vecs = np.zeros((P, 14), np.float32)
    for col, v in ((0, gq), (2, bq), (4, gk), (6, bk), (8, gv), (10, bv), (12, bout)):
        vecs[:, col] = v[:P]
        vecs[:, col + 1] = v[P:]
    wout = np.ascontiguousarray(Wout[:, :, 0, 0].T.reshape(NCHUNK, P, C)
                                .transpose(1, 0, 2)).astype(np.float32)
    idx = _rel_index()                     # [n, m]
    eb = np.exp(bias_table.astype(np.float32))[idx]   # [n, m, H]
    ebT = eb.transpose(2, 1, 0)            # [H, m, n]
    ebias = np.ascontiguousarray(ebT.reshape(HEADS, 8, P, N).astype(ml_dtypes.bfloat16))
    return {"wq": wt(Wq), "wk": wt(Wk), "wv": wt(Wv), "vecs": vecs,
            "wout": wout, "ebias": ebias}


def kernel(x, Wq, gq, bq, Wk, gk, bk, Wv, gv, bv, bias_table, Wout, bout):
    x = np.asarray(x, np.float32)
    if "nc" not in _cache:
        _cache["nc"] = build_nc()
    nc = _cache["nc"]
    shared = _prep_shared(np.asarray(Wq), np.asarray(gq), np.asarray(bq),
                          np.asarray(Wk), np.asarray(gk), np.asarray(bk),
                          np.asarray(Wv), np.asarray(gv), np.asarray(bv),
                          np.asarray(bias_table), np.asarray(Wout), np.asarray(bout))
    in_maps = []
    for b in range(B):
        m = dict(shared)
        m["x"] = np.ascontiguousarray(
            x[b].reshape(NCHUNK, P, IH, IW).astype(ml_dtypes.bfloat16))
        in_maps.append(m)
    _cache["last_in_maps"] = in_maps
    res = bass_utils.run_bass_kernel_spmd(nc, in_maps, core_ids=list(range(B)))
    out = np.stack([r["out"] for r in res.results])          # [B, 128, 2, 1024]
    out = out.transpose(0, 2, 1, 3).reshape(B, C, IH, IW)
    return np.ascontiguousarray(out.astype(np.float32))


if __name__ == "__main__":
    rng = np.random.default_rng(0)
    inputs = {
        'x': rng.standard_normal((B, C, IH, IW), dtype=np.float32),
        'Wq': (rng.standard_normal((C, C, 3, 3)) * 0.02).astype(np.float32),
        'gq': np.ones(C, np.float32), 'bq': np.zeros(C, np.float32),
        'Wk': (rng.standard_normal((C, C, 3, 3)) * 0.02).astype(np.float32),
        'gk': np.ones(C, np.float32), 'bk': np.zeros(C, np.float32),
        'Wv': (rng.standard_normal((C, C, 3, 3)) * 0.02).astype(np.float32),
        'gv': np.ones(C, np.float32), 'bv': np.zeros(C, np.float32),
        'bias_table': (rng.standard_normal(((2 * IH - 1) * (2 * IW - 1), HEADS)) * 0.02).astype(np.float32),
        'Wout': (rng.standard_normal((C, C, 1, 1)) * 0.02).astype(np.float32),
        'bout': np.zeros(C, np.float32),
    }
    out = kernel(**inputs)
    print("out", out.shape, out.dtype, np.abs(out).max())



# revision 3
# speedup vs baseline: 1.0488x; 1.0488x over previous
"""Trainium2 Bass kernel for nn_Attention_85169201480311.

Dense transformer block: 3x (conv3x3 -> GroupNorm(1) -> exact GELU) projections,
8-head attention over 1024 tokens with relative-position bias, 1x1 out-conv.

Sharding: data-parallel over batch (8 samples -> 8 cores), params replicated.

Per-core program (v2 rewrite):
 - conv3x3 = 18 PSUM-accumulating bf16 matmuls (2 cin chunks x 9 taps) per
   [P,512] output group against a zero-padded [128, 2, 34, 34] SBUF image.
   q,k evict directly from PSUM via fused GroupNorm-affine + GELU ACT; the
   rstd is computed with a DVE Newton iteration (no Sqrt table load).
 - v conv overlaps the attention exp stream: raw bf16 eviction from a 2-bank
   rotating PSUM pool, GN+GELU applied later as an SBUF->SBUF ACT pass.
 - scores in scores^T layout: per (head, i) tile one [128,1024] fp32 PSUM slot
   from a 3-slot ring; pairs of heads run concurrently in distinct PE row
   groups. exp via scalar ACT (no max-subtraction); relative-position bias
   applied multiplicatively post-exp (exp(bias) precomputed host-side, bf16).
   A configurable subset of tiles instead uses a DVE/GpSimd "fast exp"
   (Schraudolph int32-convert bit trick) to offload the scalar engine.
 - attn@v: lhsT = [v_head | ones] (M=33), psum row 32/96 carries the softmax
   denominator; two heads per psum tile via tile_position col packing. av
   matmul groups are interleaved (lagged) into the scores stream to keep the
   PE warm. Reciprocal via [128,16] DMA respread; normalizing multiply reads
   psum directly into the [c, n] layout the 1x1 out-conv consumes.
"""
import sys
for p in ('/opt/trn_rl_repo', '/root/.axon_site/_ro/trn_rl_repo'):
    if p not in sys.path:
        sys.path.insert(0, p)

import numpy as np
import ml_dtypes

import concourse.bass as bass
import concourse.tile as tile
from concourse import mybir, bacc, bass_isa
from concourse import bass_utils
from concourse.masks import make_identity

F32 = mybir.dt.float32
BF16 = mybir.dt.bfloat16
I32 = mybir.dt.int32
AF = mybir.ActivationFunctionType
ALU = mybir.AluOpType

IH = IW = 32
N = IH * IW          # 1024 tokens
C = 256
HEADS = 8
DH = 32
SCALE = 32 ** -0.5
EPS = 1e-6
B = 8
P = 128
NCHUNK = C // P      # 2 channel chunks

N_WARM = 10

# fast-exp (Schraudolph) constants: exp(SCALE*s) = 2^(A*s), A = SCALE*log2(e)
FAST_A = SCALE * 1.4426950408889634 * (1 << 23)
# bias: 127<<23 minus mantissa-linear correction (tuned for min max rel err)
FAST_B = float((127 << 23) - 486411)

# per-pair local tile indices (0..15) routed to the DVE fast-exp path
FAST_LOCAL = set()
# per-pair local tile indices whose eb-multiply runs on GpSimd
GP_LOCAL = set()
# of the fast tiles, these do pass2 (mul) on gpsimd instead of DVE
FAST_GP_LOCAL = set()

_cache = {}


def _rel_index():
    coords = np.stack(np.meshgrid(np.arange(IH), np.arange(IW), indexing='ij')).reshape(2, -1)
    rel = coords[:, :, None] - coords[:, None, :]
    rel[0] += IH - 1
    rel[1] += IW - 1
    rel[0] *= 2 * IW - 1
    return rel.sum(0)  # [n, m] int


def build_nc():
    nc = bacc.Bacc('TRN2', target_bir_lowering=False)

    x_d = nc.dram_tensor("x", [NCHUNK, P, IH, IW], BF16, kind="ExternalInput")
    w_d = {}
    for nm in ("wq", "wk", "wv"):
        w_d[nm] = nc.dram_tensor(nm, [P, NCHUNK, 9, C], BF16, kind="ExternalInput")
    wout_d = nc.dram_tensor("wout", [P, NCHUNK, C], F32, kind="ExternalInput")
    vecs_d = nc.dram_tensor("vecs", [P, 14], F32, kind="ExternalInput")
    eb_d = nc.dram_tensor("ebias", [HEADS, 8, P, N], BF16, kind="ExternalInput")
    out_d = nc.dram_tensor("out", [P, NCHUNK, N], F32, kind="ExternalOutput")

    # vecs cols: gq0,gq1,bq0,bq1, gk0,gk1,bk0,bk1, gv0,gv1,bv0,bv1, bout0,bout1
    VGCOL = {"wq": 0, "wk": 4, "wv": 8}

    with tile.TileContext(nc) as tc:
        with tc.tile_pool(name="const", bufs=1) as const, \
             tc.tile_pool(name="proj", bufs=1) as proj, \
             tc.tile_pool(name="stats", bufs=2) as stats_p, \
             tc.tile_pool(name="attn", bufs=1) as attn_p:

            xpad = const.tile([P, NCHUNK, IH + 2, IW + 2], BF16)
            nc.vector.memset(xpad[:], 0.0)
            for c in range(NCHUNK):
                nc.sync.dma_start(xpad[:, c, 1:IH + 1, 1:IW + 1], x_d[c])
            vecs = const.tile([P, 14], F32)
            nc.sync.dma_start(vecs[:], vecs_d[:])
            wout_sb = const.tile([P, NCHUNK, C], F32)
            ident = const.tile([P, P], BF16)

            q_sb = proj.tile([P, NCHUNK, N], BF16)
            k_sb = proj.tile([P, NCHUNK, N], BF16)
            v_raw = proj.tile([P, NCHUNK, N], BF16)
            v_sb = proj.tile([P, NCHUNK, N], BF16)
            v_aug = proj.tile([P, 8, 8, 34], BF16)   # [*, i, h, 0:32 v | 32 ones]
            attn_out = attn_p.tile([P, NCHUNK, N], F32)
            out_sb = attn_p.tile([P, NCHUNK, N], F32)
            nc.vector.memset(v_aug[:, :, :, 32:33], 1.0)

            w_sb = {}

            # ---------- GroupNorm stats chain (shared by q,k,v) ----------
            def gn_chain(nm, st_t):
                """st_t: [P,2,6] bn_stats per m-chunk. Returns sc [P,4]:
                cols 0,1 = scale per m; 2,3 = bias per m."""
                mv = [stats_p.tile([P, 2], F32, name=f"mv_{nm}_{m}", tag="mv")
                      for m in range(2)]
                for m in range(2):
                    nc.vector.bn_aggr(out=mv[m][:], in_=st_t[m][:])
                prep = stats_p.tile([P, 4], F32, name=f"prep_{nm}", tag="prep")
                for m in range(2):
                    nc.vector.tensor_copy(out=prep[:, 2 * m:2 * m + 1], in_=mv[m][:, 0:1])
                    sq = stats_p.tile([P, 1], F32, name=f"sq_{nm}_{m}", tag="sq")
                    nc.vector.tensor_mul(out=sq[:], in0=mv[m][:, 0:1], in1=mv[m][:, 0:1])
                    nc.vector.tensor_add(out=prep[:, 2 * m + 1:2 * m + 2],
                                         in0=mv[m][:, 1:2], in1=sq[:])
                red = stats_p.tile([P, 4], F32, name=f"red_{nm}", tag="red")
                nc.gpsimd.partition_all_reduce(red[:], prep[:], channels=P,
                                               reduce_op=bass_isa.ReduceOp.add)
                # mean = (c0+c2)/256 ; var = (c1+c3)/256 - mean^2
                mt = stats_p.tile([P, 4], F32, name=f"mt_{nm}", tag="mt")
                nc.vector.tensor_add(out=mt[:, 0:1], in0=red[:, 0:1], in1=red[:, 2:3])
                nc.scalar.mul(out=mt[:, 0:1], in_=mt[:, 0:1], mul=1.0 / C)
                nc.vector.tensor_add(out=mt[:, 1:2], in0=red[:, 1:2], in1=red[:, 3:4])
                nc.scalar.mul(out=mt[:, 1:2], in_=mt[:, 1:2], mul=1.0 / C)
                nc.vector.tensor_mul(out=mt[:, 2:3], in0=mt[:, 0:1], in1=mt[:, 0:1])
                nc.vector.tensor_sub(out=mt[:, 1:2], in0=mt[:, 1:2], in1=mt[:, 2:3])
                # rstd = 1/sqrt(var+eps) via reciprocal + Newton (no Sqrt table)
                ve = stats_p.tile([P, 1], F32, name=f"ve_{nm}", tag="ve")
                nc.vector.tensor_scalar_add(out=ve[:], in0=mt[:, 1:2], scalar1=EPS)
                y = stats_p.tile([P, 1], F32, name=f"y_{nm}", tag="y")
                nc.vector.reciprocal(out=y[:], in_=ve[:])
                # seed y0 = 0.5*(1 + 1/v)  (good for v in ~[0.2, 5])
                nc.vector.tensor_scalar(out=y[:], in0=y[:], scalar1=0.5, scalar2=0.5,
                                        op0=ALU.mult, op1=ALU.add)
                t1 = stats_p.tile([P, 1], F32, name=f"t1_{nm}", tag="t1")
                for _ in range(4):
                    nc.vector.tensor_mul(out=t1[:], in0=y[:], in1=y[:])
                    nc.vector.tensor_mul(out=t1[:], in0=t1[:], in1=ve[:])
                    nc.vector.tensor_scalar(out=t1[:], in0=t1[:], scalar1=-0.5,
                                            scalar2=1.5, op0=ALU.mult, op1=ALU.add)
                    nc.vector.tensor_mul(out=y[:], in0=y[:], in1=t1[:])
                gc = VGCOL[nm]
                sc = stats_p.tile([P, 4], F32, name=f"sc_{nm}", tag="sc")
                for m in range(2):
                    # s_m = g_m * rstd ; t_m = b_m - mean * s_m
                    nc.vector.tensor_mul(out=sc[:, m:m + 1],
                                         in0=vecs[:, gc + m:gc + m + 1], in1=y[:])
                    nc.vector.tensor_mul(out=sc[:, 2 + m:3 + m],
                                         in0=mt[:, 0:1], in1=sc[:, m:m + 1])
                    nc.vector.tensor_sub(out=sc[:, 2 + m:3 + m],
                                         in0=vecs[:, gc + 2 + m:gc + 3 + m],
                                         in1=sc[:, 2 + m:3 + m])
                return sc

            def conv_group(psum_t, nm, m, j):
                first = True
                for c in range(NCHUNK):
                    for t in range(9):
                        dy, dx = t // 3, t % 3
                        rhs = xpad[:, c, 16 * j + dy:16 * j + dy + 16, dx:dx + 32]
                        nc.tensor.matmul(
                            psum_t[:], w_sb[nm][:, c, t, m * P:(m + 1) * P], rhs,
                            start=first, stop=(c == NCHUNK - 1 and t == 8))
                        first = False

            # ---------------- q,k conv (direct GELU eviction) ----------------
            with tc.tile_pool(name="wpool", bufs=1) as wpool, \
                 tc.tile_pool(name="psc", bufs=1, space="PSUM") as psc:
                warm_sb = wpool.tile([P, 512], BF16)
                nc.vector.memset(warm_sb[:], 0.0)
                ps_w = psc.tile([P, 512], F32, name="warm_ps", tag="warm", bufs=1)
                for _ in range(N_WARM):
                    nc.tensor.matmul(ps_w[:], warm_sb[:, 0:P], warm_sb[:],
                                     start=True, stop=True)
                for nm in ("wq", "wk", "wv"):
                    w_sb[nm] = wpool.tile([P, NCHUNK, 9, C], BF16, name=f"sb_{nm}")
                    nc.sync.dma_start(w_sb[nm][:], w_d[nm][:])
                nc.sync.dma_start(wout_sb[:], wout_d[:])
                make_identity(nc, ident[:])

                for nm, dst in (("wq", q_sb), ("wk", k_sb)):
                    ps_t = [psc.tile([P, 512], F32, name=f"cv_{nm}_{m}_{j}",
                                     tag="cv", bufs=5)
                            for m in range(2) for j in range(2)]
                    st_t = [stats_p.tile([P, 2, 6], F32, name=f"st_{nm}_{m}", tag="st")
                            for m in range(2)]
                    for m in range(2):
                        for j in range(2):
                            conv_group(ps_t[m * 2 + j], nm, m, j)
                            nc.vector.bn_stats(out=st_t[m][:, j, :], in_=ps_t[m * 2 + j][:])
                    sc = gn_chain(nm, st_t)
                    for m in range(2):
                        for j in range(2):
                            nc.scalar.activation(
                                out=dst[:, m, 512 * j:512 * (j + 1)],
                                in_=ps_t[m * 2 + j][:],
                                func=AF.Gelu, scale=sc[:, m:m + 1], bias=sc[:, 2 + m:3 + m])

            # ---------------- attention (+ overlapped v conv) ----------------
            from contextlib import ExitStack
            with ExitStack() as stk:
                ring_p = stk.enter_context(tc.tile_pool(name="ring_p", bufs=3,
                                                        space="PSUM"))
                vv_ctx = ExitStack()
                vv_p = vv_ctx.enter_context(tc.tile_pool(name="vv_p", bufs=2,
                                                         space="PSUM"))
                attnT_p = stk.enter_context(tc.tile_pool(name="attnT_p", bufs=4))
                ebuf = stk.enter_context(tc.tile_pool(name="ebuf", bufs=3))
                ebias_p = stk.enter_context(tc.tile_pool(name="ebias_p", bufs=12))
                fast_p = stk.enter_context(tc.tile_pool(name="fast_p", bufs=3))
                rc_p = stk.enter_context(tc.tile_pool(name="rc", bufs=2))

                attnTs = {}
                eb_tiles = {}
                ring_n = [0]
                st_v = [stats_p.tile([P, 2, 6], F32, name=f"st_v_{m}", tag="st")
                        for m in range(2)]
                av_state = {}

                def issue_eb(p, i):
                    for h in (2 * p, 2 * p + 1):
                        t = ebias_p.tile([P, N], BF16, name=f"eb_{h}_{i}", tag="eb")
                        nc.sync.dma_start(t[:], eb_d[h, i])
                        eb_tiles[(h, i)] = t

                def v_half_group(hg):
                    # half-group hg in 0..7: group (m, j) = hg//2, taps half hg%2
                    m, j = hg // 4, (hg // 2) % 2
                    half = hg % 2
                    key = (m, j)
                    if half == 0:
                        av_state[key] = vv_p.tile([P, 512], F32,
                                                  name=f"cvv_{m}_{j}", tag="vv")
                    pt = av_state[key]
                    items = [(c, t) for c in range(NCHUNK) for t in range(9)]
                    items = items[:9] if half == 0 else items[9:]
                    for idx, (c, t) in enumerate(items):
                        dy, dx = t // 3, t % 3
                        rhs = xpad[:, c, 16 * j + dy:16 * j + dy + 16, dx:dx + 32]
                        nc.tensor.matmul(pt[:], w_sb["wv"][:, c, t, m * P:(m + 1) * P],
                                         rhs, start=(half == 0 and idx == 0),
                                         stop=(half == 1 and idx == 8))
                    if half == 1:
                        nc.vector.bn_stats(out=st_v[m][:, j, :], in_=pt[:])
                        nc.vector.tensor_copy(
                            out=v_raw[:, m, 512 * j:512 * (j + 1)], in_=pt[:])

                def scores_slot(p, i):
                    h0, h1 = 2 * p, 2 * p + 1
                    rts = {}
                    for h in (h0, h1):
                        rts[h] = ring_p.tile([P, N], F32, name=f"s_{h}_{i}", tag="ring")
                    for nj in range(2):
                        for h in (h0, h1):
                            g, r = h // 4, h % 4
                            nc.tensor.matmul(
                                rts[h][:, 512 * nj:512 * (nj + 1)],
                                k_sb[32 * r:32 * r + 32, g, P * i:P * (i + 1)],
                                q_sb[32 * r:32 * r + 32, g, 512 * nj:512 * (nj + 1)],
                                start=True, stop=True, tile_position=(32 * r, 0))
                    return rts

                def exp_tile(p, i, h, rt):
                    l = (i % 8) * 2 + (h & 1)
                    eb_t = eb_tiles.pop((h, i))
                    dst = attnTs[h][:, i, :]
                    if l in FAST_LOCAL:
                        ti = fast_p.tile([P, N], I32, name=f"f_{h}_{i}", tag="f")
                        nc.vector.tensor_scalar(out=ti[:], in0=rt[:],
                                                scalar1=FAST_A, scalar2=FAST_B,
                                                op0=ALU.mult, op1=ALU.add)
                        eng = nc.gpsimd if l in FAST_GP_LOCAL else nc.vector
                        eng.scalar_tensor_tensor(
                            out=dst, in0=ti[:].bitcast(F32), scalar=1.0,
                            in1=eb_t[:], op0=ALU.mult, op1=ALU.mult)
                    else:
                        e_bf = ebuf.tile([P, N], BF16, name=f"e_{h}_{i}", tag="e")
                        nc.scalar.activation(out=e_bf[:], in_=rt[:], func=AF.Exp,
                                             scale=SCALE)
                        if l in GP_LOCAL:
                            nc.gpsimd.scalar_tensor_tensor(
                                out=dst, in0=e_bf[:], scalar=1.0, in1=eb_t[:],
                                op0=ALU.mult, op1=ALU.mult)
                        else:
                            nc.vector.tensor_mul(out=dst, in0=e_bf[:], in1=eb_t[:])

                def av_unit(p, i):
                    h0, h1 = 2 * p, 2 * p + 1
                    if i == 0:
                        av_state['ps'] = av_pool.tile([P, N], F32,
                                                      name=f"av_{p}", tag="av")
                    ps_av = av_state['ps']
                    for nj in range(2):
                        for h in (h0, h1):
                            rv = h % 2
                            nc.tensor.matmul(
                                ps_av[64 * rv:64 * rv + 33, 512 * nj:512 * (nj + 1)],
                                v_aug[:, i, h, 0:33],
                                attnTs[h][:, i, 512 * nj:512 * (nj + 1)],
                                start=(i == 0), stop=(i == 7),
                                tile_position=(0, 64 * rv))
                    if i == 7:
                        av_norm(p, ps_av)

                def av_norm(p, ps_av):
                    grp = p // 2
                    h0, h1 = 2 * p, 2 * p + 1
                    srows = rc_p.tile([64, N], F32, name=f"sr_{p}", tag="sr")
                    sp = rc_p.tile([P, 16], F32, name=f"sp_{p}", tag="sp")
                    rrow = rc_p.tile([64, N], F32, name=f"rr_{p}", tag="rr")
                    rcp_bc = rc_p.tile([P, N], F32, name=f"rb_{p}", tag="rb")
                    for rv in range(2):
                        nc.vector.tensor_copy(out=srows[32 * rv:32 * rv + 1, :],
                                              in_=ps_av[64 * rv + 32:64 * rv + 33, :])
                        nc.sync.dma_start(out=sp[:, 8 * rv:8 * rv + 8],
                                          in_=srows[32 * rv:32 * rv + 1, :])
                    nc.vector.reciprocal(out=sp[:], in_=sp[:])
                    for rv in range(2):
                        nc.sync.dma_start(out=rrow[32 * rv:32 * rv + 1, :],
                                          in_=sp[:, 8 * rv:8 * rv + 8])
                        rowap = rrow[32 * rv:32 * rv + 1, :]
                        src = bass.AP(tensor=rowap.tensor, offset=rowap.offset,
                                      ap=[list(rowap.ap[0]), [0, 32]]
                                      + [list(d) for d in rowap.ap[1:]])
                        nc.sync.dma_start(out=rcp_bc[64 * rv:64 * rv + 32, :], in_=src)
                    for h in (h0, h1):
                        r, rv = h % 4, h % 2
                        nc.vector.tensor_mul(out=attn_out[32 * r:32 * r + 32, grp, :],
                                             in0=ps_av[64 * rv:64 * rv + 32, :],
                                             in1=rcp_bc[64 * rv:64 * rv + 32, :])

                # ---- main interleaved attention loop ----
                V_DONE_SLOT = 9
                av_pool = None
                av_next = [0]   # next av unit (flat slot index) to emit

                def emit_av(upto, cap=3):
                    n = 0
                    while av_next[0] <= upto and n < cap:
                        cs = av_next[0]
                        av_unit(cs // 8, cs % 8)
                        av_next[0] += 1
                        n += 1

                issue_eb(0, 0)
                issue_eb(0, 1)
                issue_eb(0, 2)
                issue_eb(0, 3)
                for s in range(32):
                    p, i = s // 8, s % 8
                    if i == 0:
                        for h in (2 * p, 2 * p + 1):
                            attnTs[h] = attnT_p.tile([P, 8, N], BF16,
                                                     name=f"attnT_{h}", tag="attnT")
                    nx = s + 4
                    if nx < 32:
                        issue_eb(nx // 8, nx % 8)
                    rts = scores_slot(p, i)
                    if s < 8:
                        v_half_group(s)
                    for h in (2 * p, 2 * p + 1):
                        exp_tile(p, i, h, rts[h])
                    if s == V_DONE_SLOT:
                        # v GN+GELU pass (sbuf->sbuf), transposes, open av pool
                        sc_v = gn_chain("wv", st_v)
                        for m in range(2):
                            nc.scalar.activation(
                                out=v_sb[:, m, :], in_=v_raw[:, m, :],
                                func=AF.Gelu, scale=sc_v[:, m:m + 1],
                                bias=sc_v[:, 2 + m:3 + m])
                        vv_ctx.close()
                        with tc.tile_pool(name="pvt_p", bufs=2, space="PSUM") as pvt_p:
                            for kk in range(NCHUNK):
                                for ii in range(8):
                                    pvt = pvt_p.tile([P, P], BF16,
                                                     name=f"vt_{kk}_{ii}", tag="pvt")
                                    nc.tensor.transpose(
                                        pvt[:], v_sb[:, kk, P * ii:P * (ii + 1)],
                                        ident[:])
                                    nc.vector.tensor_copy(
                                        out=v_aug[:, ii, 4 * kk:4 * kk + 4, 0:32],
                                        in_=pvt[:])
                        av_pool = stk.enter_context(
                            tc.tile_pool(name="av_p", bufs=1, space="PSUM"))
                    elif s > V_DONE_SLOT:
                        emit_av(s - 2)
                # drain remaining av units
                emit_av(31, cap=32)

                # ---------------- 1x1 out conv (psum slots from ring) ----------------
                for m in range(2):
                    ps_o = ring_p.tile([P, N], F32, name=f"o_{m}", tag="ring")
                    for j in range(2):
                        for kk in range(NCHUNK):
                            nc.tensor.matmul(ps_o[:, 512 * j:512 * (j + 1)],
                                             wout_sb[:, kk, m * P:(m + 1) * P],
                                             attn_out[:, kk, 512 * j:512 * (j + 1)],
                                             start=(kk == 0), stop=(kk == NCHUNK - 1))
                    nc.scalar.activation(out=out_sb[:, m, :], in_=ps_o[:],
                                         func=AF.Identity,
                                         bias=vecs[:, 12 + m:13 + m], scale=1.0)
                    nc.sync.dma_start(out_d[:, m], out_sb[:, m, :])

    nc.compile()
    return nc


def _prep_shared(Wq, gq, bq, Wk, gk, bk, Wv, gv, bv, bias_table, Wout, bout):
    def wt(W):
        # [co, ci, 3, 3] -> [ci%128, ci//128, tap, co]
        return np.ascontiguousarray(
            W.astype(np.float32).transpose(1, 2, 3, 0).reshape(NCHUNK, P, 9, C)
            .transpose(1, 0, 2, 3)).astype(ml_dtypes.bfloat16)
    vecs = np.zeros((P, 14), np.float32)
    for col, v in ((0, gq), (2, bq), (4, gk), (6, bk), (8, gv), (10, bv), (12, bout)):
        vecs[:, col] = v[:P]
        vecs[:, col + 1] = v[P:]
    wout = np.ascontiguousarray(Wout[:, :, 0, 0].T.reshape(NCHUNK, P, C)
                                .transpose(1, 0, 2)).astype(np.float32)
    idx = _rel_index()                     # [n, m]
    eb = np.exp(bias_table.astype(np.float32))[idx]   # [n, m, H]
    ebT = eb.transpose(2, 1, 0)            # [H, m, n]
    ebias = np.ascontiguousarray(ebT.reshape(HEADS, 8, P, N).astype(ml_dtypes.bfloat16))
    return {"wq": wt(Wq), "wk": wt(Wk), "wv": wt(Wv), "vecs": vecs,
            "wout": wout, "ebias": ebias}


def kernel(x, Wq, gq, bq, Wk, gk, bk, Wv, gv, bv, bias_table, Wout, bout):
    x = np.asarray(x, np.float32)
    if "nc" not in _cache:
        _cache["nc"] = build_nc()
    nc = _cache["nc"]
    shared = _prep_shared(np.asarray(Wq), np.asarray(gq), np.asarray(bq),
                          np.asarray(Wk), np.asarray(gk), np.asarray(bk),
                          np.asarray(Wv), np.asarray(gv), np.asarray(bv),
                          np.asarray(bias_table), np.asarray(Wout), np.asarray(bout))
    in_maps = []
    for b in range(B):
        m = dict(shared)
        m["x"] = np.ascontiguousarray(
            x[b].reshape(NCHUNK, P, IH, IW).astype(ml_dtypes.bfloat16))
        in_maps.append(m)
    _cache["last_in_maps"] = in_maps
    res = bass_utils.run_bass_kernel_spmd(nc, in_maps, core_ids=list(range(B)))
    out = np.stack([r["out"] for r in res.results])          # [B, 128, 2, 1024]
    out = out.transpose(0, 2, 1, 3).reshape(B, C, IH, IW)
    return np.ascontiguousarray(out.astype(np.float32))


if __name__ == "__main__":
    rng = np.random.default_rng(0)
    inputs = {
        'x': rng.standard_normal((B, C, IH, IW), dtype=np.float32),
        'Wq': (rng.standard_normal((C, C, 3, 3)) * 0.02).astype(np.float32),
        'gq': np.ones(C, np.float32), 'bq': np.zeros(C, np.float32),
        'Wk': (rng.standard_normal((C, C, 3, 3)) * 0.02).astype(np.float32),
        'gk': np.ones(C, np.float32), 'bk': np.zeros(C, np.float32),
        'Wv': (rng.standard_normal((C, C, 3, 3)) * 0.02).astype(np.float32),
        'gv': np.ones(C, np.float32), 'bv': np.zeros(C, np.float32),
        'bias_table': (rng.standard_normal(((2 * IH - 1) * (2 * IW - 1), HEADS)) * 0.02).astype(np.float32),
        'Wout': (rng.standard_normal((C, C, 1, 1)) * 0.02).astype(np.float32),
        'bout': np.zeros(C, np.float32),
    }
    out = kernel(**inputs)
    print("out", out.shape, out.dtype, np.abs(out).max())
